# revision 1
# baseline (speedup 1.0000x reference)
"""CausalFlowModel Trainium2 kernel: LSTM-style scan over T steps, batch-sharded on 8 cores.

Layout: feature-major ([z-feature partitions, batch free-dim]).
State h is stored as h/2 (folded into weights); tanh via 2*sigmoid(2x)-1.
"""
import sys

sys.path.insert(0, "/opt/trn_rl_repo")
import numpy as np
import concourse.bass as bass
import concourse.bacc as bacc
import concourse.mybir as mybir
from concourse.tile import TileContext
from concourse.bass_utils import run_bass_kernel_spmd

F32 = mybir.dt.float32
AF = mybir.ActivationFunctionType
ALU = mybir.AluOpType

N_CORES = 8
STATE, CTRL, RNN, OUT = 16, 8, 64, 16
Z = RNN + STATE  # 80
KU = CTRL + 1  # 9
KAUG = Z + KU + 1  # 90
X_DNN_H = 2 * RNN  # 128
U_DNN_H = 2 * Z  # 160

# our z-feature order: [h0 (64); x (16)]  (z_ours[i] = z_ref[ZPERM[i]])
ZPERM = np.concatenate([np.arange(STATE, Z), np.arange(0, STATE)])


def _np(a):
    return np.asarray(a, dtype=np.float32)


def prep(x, rnn_input, deltas, lengths, x_dnn_params, u_dnn_params, W_ih, W_hh, b_rnn):
    """Host-side data marshaling. Returns (in_maps, meta)."""
    B, T = rnn_input.shape[0], rnn_input.shape[1]
    assert B % N_CORES == 0
    S = B // N_CORES
    x = _np(x); rnn_input = _np(rnn_input); deltas = _np(deltas)
    lengths = np.asarray(lengths, dtype=np.int64)

    order = np.argsort(-lengths, kind="stable")  # descending
    idx = order.reshape(S, N_CORES)  # idx[:, j] = core j's columns (desc sorted)

    # per-step active width (max over cores, padded to mult of 4)
    lens_sorted = lengths[order]
    percore = lens_sorted.reshape(S, N_CORES)
    Ns = []
    for t in range(T):
        n = int((percore > t).sum(axis=0).max())
        n = min(S, max(4, -(-n // 4) * 4))
        Ns.append(n)

    # weights (shared across cores)
    (W1, b1), (W2, b2), (W3, b3) = [(_np(w), _np(b)) for (w, b) in x_dnn_params]
    (W4, b4), (W5, b5), (W6, b6) = [(_np(w), _np(b)) for (w, b) in u_dnn_params]
    W_ih = _np(W_ih); W_hh = _np(W_hh); b_rnn = _np(b_rnn)

    colperm = np.concatenate([g * Z + ZPERM for g in range(4)])
    Whh_p = W_hh[np.ix_(ZPERM, colperm)]
    Wih_p = W_ih[:, colperm]
    b_p = b_rnn[colperm]
    Waug = np.concatenate([2.0 * Whh_p, Wih_p, b_p[None, :]], axis=0)  # [90, 320]
    Waug[:, 2 * Z:3 * Z] *= 2.0  # sigma(2g) trick on the g gate
    Waug = np.ascontiguousarray(Waug, dtype=np.float32)

    W3h = np.ascontiguousarray(W3 * 0.5)  # h0/2
    b3h = np.ascontiguousarray((b3 * 0.5)[:, None])
    W4e = np.ascontiguousarray(2.0 * W4[ZPERM])  # consumes e/2 in permuted order

    shared = {
        "Wx1": W1, "bx1": np.ascontiguousarray(b1[:, None]),
        "Wx2": W2, "bx2": np.ascontiguousarray(b2[:, None]),
        "Wx3": W3h, "bx3": b3h,
        "Waug": Waug,
        "Wu1": W4e,
        "bu1a": np.ascontiguousarray(b4[:128, None]),
        "bu1b": np.ascontiguousarray(b4[128:, None]),
        "Wu2a": np.ascontiguousarray(W5[:128]), "Wu2b": np.ascontiguousarray(W5[128:]),
        "bu2a": np.ascontiguousarray(b5[:128, None]),
        "bu2b": np.ascontiguousarray(b5[128:, None]),
        "Wu3a": np.ascontiguousarray(W6[:128]), "Wu3b": np.ascontiguousarray(W6[128:]),
        "bu3": np.ascontiguousarray(b6[:, None]),
    }

    tgrid = np.arange(T)
    in_maps = []
    for j in range(N_CORES):
        sel = idx[:, j]
        lj = lengths[sel]  # [S]
        xT_half = np.ascontiguousarray(0.5 * x[sel].T)  # [16, S]
        uT = np.ascontiguousarray(rnn_input[sel].transpose(1, 2, 0))  # [T, 9, S]
        dT = deltas[sel, :, 0].T.copy()  # [T, S]
        dT[tgrid[:, None] >= lj[None, :]] = 0.0  # freeze past end-of-sequence
        dT = np.ascontiguousarray(dT)
        dlast = np.ascontiguousarray(deltas[sel, lj - 1, 0][None, :])  # [1, S]
        maskA = np.ascontiguousarray(  # 1.0 where L odd: h_last in buf0 / h_prev in buf1
            np.broadcast_to(((lj % 2) == 1).astype(np.float32)[None, :], (Z, S)))
        m = dict(shared)
        m.update({"xT": xT_half, "uT": uT, "dT": dT, "dlast": dlast, "maskA": maskA})
        in_maps.append(m)

    meta = {"S": S, "T": T, "Ns": Ns, "idx": idx}
    return in_maps, meta


def build(S, T, Ns):
    nc = bacc.Bacc("TRN2", target_bir_lowering=False, debug=False,
                   num_devices=N_CORES)
    d = {}
    def din(name, shape):
        d[name] = nc.dram_tensor(name, list(shape), F32, kind="ExternalInput")
    din("xT", (STATE, S)); din("uT", (T, KU, S)); din("dT", (T, S))
    din("dlast", (1, S)); din("maskA", (Z, S))
    din("Wx1", (STATE, X_DNN_H)); din("bx1", (X_DNN_H, 1))
    din("Wx2", (X_DNN_H, X_DNN_H)); din("bx2", (X_DNN_H, 1))
    din("Wx3", (X_DNN_H, RNN)); din("bx3", (RNN, 1))
    din("Waug", (KAUG, 4 * Z))
    din("Wu1", (Z, U_DNN_H)); din("bu1a", (128, 1)); din("bu1b", (U_DNN_H - 128, 1))
    din("Wu2a", (128, U_DNN_H)); din("Wu2b", (U_DNN_H - 128, U_DNN_H))
    din("bu2a", (128, 1)); din("bu2b", (U_DNN_H - 128, 1))
    din("Wu3a", (128, OUT)); din("Wu3b", (U_DNN_H - 128, OUT))
    din("bu3", (OUT, 1))
    outT = nc.dram_tensor("outT", [OUT, S], F32, kind="ExternalOutput")

    NUB = 14  # steps per u-store group (14*9=126 partitions)
    n_ugrp = -(-T // NUB)

    with TileContext(nc) as tc:
        with (
            tc.tile_pool(name="const", bufs=1) as cp,
            tc.tile_pool(name="state", bufs=1) as sp,
            tc.tile_pool(name="stage", bufs=4) as stp,
            tc.tile_pool(name="psum", bufs=1, space="PSUM") as pp,
            tc.tile_pool(name="psumd", bufs=2, space="PSUM") as ppd,
        ):
            # ---- load constants ----
            def load(name, shape):
                t = cp.tile(list(shape), F32, tag=name)
                nc.sync.dma_start(out=t[...], in_=d[name][...])
                return t
            Wx1 = load("Wx1", (STATE, X_DNN_H)); bx1 = load("bx1", (X_DNN_H, 1))
            Wx2 = load("Wx2", (X_DNN_H, X_DNN_H)); bx2 = load("bx2", (X_DNN_H, 1))
            Wx3 = load("Wx3", (X_DNN_H, RNN)); bx3 = load("bx3", (RNN, 1))
            Waug = load("Waug", (KAUG, 4 * Z))
            Wu1 = load("Wu1", (Z, U_DNN_H))
            bu1a = load("bu1a", (128, 1)); bu1b = load("bu1b", (U_DNN_H - 128, 1))
            Wu2a = load("Wu2a", (128, U_DNN_H)); Wu2b = load("Wu2b", (U_DNN_H - 128, U_DNN_H))
            bu2a = load("bu2a", (128, 1)); bu2b = load("bu2b", (U_DNN_H - 128, 1))
            Wu3a = load("Wu3a", (128, OUT)); Wu3b = load("Wu3b", (U_DNN_H - 128, OUT))
            bu3 = load("bu3", (OUT, 1))
            maskA = load("maskA", (Z, S))
            xsb = load("xT", (STATE, S))

            ustore = cp.tile([NUB * KU, n_ugrp, S], F32, tag="ustore")
            for k in range(n_ugrp):
                t0, t1 = k * NUB, min((k + 1) * NUB, T)
                nc.sync.dma_start(
                    out=ustore[: (t1 - t0) * KU, k, :],
                    in_=d["uT"][t0:t1, :, :].rearrange("a b c -> (a b) c"),
                )

            ones = cp.tile([1, Z], F32, tag="ones")
            nc.vector.memset(ones[...], 1.0)

            # ---- state tiles ----
            rhsA = sp.tile([KAUG, S], F32, tag="rhsA")
            rhsB = sp.tile([KAUG, S], F32, tag="rhsB")
            rhs = [rhsA, rhsB]
            c = sp.tile([Z, S], F32, tag="c")
            sg = sp.tile([Z, 4, S], F32, tag="sg")
            sc = sp.tile([Z, S], F32, tag="sc")
            a2t = sp.tile([Z, S], F32, tag="a2t")
            t2t = sp.tile([Z, S], F32, tag="t2t")
            ht2t = sp.tile([Z, S], F32, tag="ht2t")
            dltt = sp.tile([Z, S], F32, tag="dltt")
            mt = sp.tile([Z, S], F32, tag="mt")
            htmp = sp.tile([Z, S], F32, tag="htmp")

            nc.vector.memset(c[...], 0.0)
            onesrow = cp.tile([1, S], F32, tag="onesrow")
            nc.vector.memset(onesrow[...], 1.0)
            nc.sync.dma_start(out=rhsA[Z + KU:, :], in_=onesrow[...])
            nc.sync.dma_start(out=rhsB[Z + KU:, :], in_=onesrow[...])

            # ---- encoder: z/2 -> rhsB ----
            ep = pp.tile([X_DNN_H, S], F32, tag="mm128")
            nc.tensor.matmul(ep[...], Wx1[...], xsb[...], start=True, stop=True)
            e1 = sp.tile([X_DNN_H, S], F32, tag="enc1")
            nc.scalar.activation(e1[...], ep[...], AF.Tanh, bias=bx1[:, 0:1])
            ep2 = pp.tile([X_DNN_H, S], F32, tag="mm128")
            nc.tensor.matmul(ep2[...], Wx2[...], e1[...], start=True, stop=True)
            e2 = sp.tile([X_DNN_H, S], F32, tag="enc2")
            nc.scalar.activation(e2[...], ep2[...], AF.Tanh, bias=bx2[:, 0:1])
            ep3 = pp.tile([RNN, S], F32, tag="small")
            nc.tensor.matmul(ep3[...], Wx3[...], e2[...], start=True, stop=True)
            nc.scalar.activation(rhsB[:RNN, :], ep3[...], AF.Identity, bias=bx3[:, 0:1])
            # x/2 into partitions 64:80 of rhsB (already halved on host)
            nc.sync.dma_start(out=rhsB[RNN:Z, :], in_=xsb[...])

            gates = pp.tile([Z, 4, S], F32, tag="gates")

            # ---- the scan ----
            for t in range(T):
                N = Ns[t]
                cur = rhs[(t + 1) % 2]
                nxt = rhs[t % 2]
                # u_t into rhs rows 80:89 (prefetchable)
                nc.sync.dma_start(
                    out=cur[Z:Z + KU, :N],
                    in_=ustore[(t % NUB) * KU:(t % NUB + 1) * KU, t // NUB, :N],
                )
                # d_t staged + broadcast via ones-matmul
                stg = stp.tile([1, S], F32, tag="dstage")
                nc.sync.dma_start(out=stg[:, :N], in_=d["dT"][t:t + 1, :N])
                dbc = ppd.tile([Z, S], F32, tag="dbc")
                nc.tensor.matmul(dbc[:, :N], ones[...], stg[:, :N], start=True, stop=True)
                # gates
                for g in range(4):
                    nc.tensor.matmul(gates[:, g, :N], Waug[:, g * Z:(g + 1) * Z],
                                     cur[:, :N], start=True, stop=True)
                nc.scalar.activation(sg[:, :, :N], gates[:, :, :N], AF.Sigmoid)
                # c' = f*c + i*tanh(g)   (all in half-units)
                nc.vector.scalar_tensor_tensor(
                    a2t[:, :N], sg[:, 2, :N], 0.5, sg[:, 0, :N],
                    op0=ALU.subtract, op1=ALU.mult)
                nc.vector.tensor_mul(t2t[:, :N], sg[:, 1, :N], c[:, :N])
                nc.vector.tensor_add(c[:, :N], a2t[:, :N], t2t[:, :N])
                nc.scalar.activation(sc[:, :N], c[:, :N], AF.Sigmoid, scale=4.0)
                # h' = h + d*(htilde - h)
                nc.vector.scalar_tensor_tensor(
                    ht2t[:, :N], sc[:, :N], 0.5, sg[:, 3, :N],
                    op0=ALU.subtract, op1=ALU.mult)
                nc.vector.tensor_sub(dltt[:, :N], ht2t[:, :N], cur[:Z, :N])
                nc.vector.tensor_mul(mt[:, :N], dbc[:, :N], dltt[:, :N])
                nc.vector.tensor_add(htmp[:, :N], cur[:Z, :N], mt[:, :N])
                # predicated write keeps retired columns frozen in BOTH ping-pong
                # buffers (mask = d broadcast, zeroed past end-of-sequence)
                nc.vector.copy_predicated(
                    nxt[:Z, :N], dbc[:, :N].bitcast(I32), htmp[:, :N])

            # ---- post-loop: e = hprev + d_last*(hlast - hprev) ----
            hlast = sp.tile([Z, S], F32, tag="hlast")
            hprev = sp.tile([Z, S], F32, tag="hprev")
            nc.vector.tensor_copy(hlast[...], rhsB[:Z, :])
            nc.vector.copy_predicated(hlast[...], maskA[...], rhsA[:Z, :])
            nc.vector.tensor_copy(hprev[...], rhsA[:Z, :])
            nc.vector.copy_predicated(hprev[...], maskA[...], rhsB[:Z, :])
            stg2 = stp.tile([1, S], F32, tag="dstage")
            nc.sync.dma_start(out=stg2[...], in_=d["dlast"][...])
            dbc2 = ppd.tile([Z, S], F32, tag="dbc")
            nc.tensor.matmul(dbc2[...], ones[...], stg2[...], start=True, stop=True)
            et = sp.tile([Z, S], F32, tag="et")
            nc.vector.tensor_sub(a2t[...], hlast[...], hprev[...])
            nc.vector.tensor_mul(t2t[...], dbc2[...], a2t[...])
            nc.vector.tensor_add(et[...], hprev[...], t2t[...])

            # ---- decoder ----
            p1a = pp.tile([128, S], F32, tag="mm128")
            nc.tensor.matmul(p1a[...], Wu1[:, :128], et[...], start=True, stop=True)
            d1a = sp.tile([128, S], F32, tag="d1a")
            nc.scalar.activation(d1a[...], p1a[...], AF.Tanh, bias=bu1a[:, 0:1])
            p1b = pp.tile([U_DNN_H - 128, S], F32, tag="small")
            nc.tensor.matmul(p1b[...], Wu1[:, 128:], et[...], start=True, stop=True)
            d1b = sp.tile([U_DNN_H - 128, S], F32, tag="d1b")
            nc.scalar.activation(d1b[...], p1b[...], AF.Tanh, bias=bu1b[:, 0:1])

            p2a = pp.tile([128, S], F32, tag="mm128")
            nc.tensor.matmul(p2a[...], Wu2a[:, :128], d1a[...], start=True, stop=False)
            nc.tensor.matmul(p2a[...], Wu2b[:, :128], d1b[...], start=False, stop=True)
            d2a = sp.tile([128, S], F32, tag="d2a")
            nc.scalar.activation(d2a[...], p2a[...], AF.Tanh, bias=bu2a[:, 0:1])
            p2b = pp.tile([U_DNN_H - 128, S], F32, tag="small")
            nc.tensor.matmul(p2b[...], Wu2a[:, 128:], d1a[...], start=True, stop=False)
            nc.tensor.matmul(p2b[...], Wu2b[:, 128:], d1b[...], start=False, stop=True)
            d2b = sp.tile([U_DNN_H - 128, S], F32, tag="d2b")
            nc.scalar.activation(d2b[...], p2b[...], AF.Tanh, bias=bu2b[:, 0:1])

            p3 = pp.tile([OUT, S], F32, tag="small")
            nc.tensor.matmul(p3[...], Wu3a[...], d2a[...], start=True, stop=False)
            nc.tensor.matmul(p3[...], Wu3b[...], d2b[...], start=False, stop=True)
            osb = sp.tile([OUT, S], F32, tag="osb")
            nc.scalar.activation(osb[...], p3[...], AF.Identity, bias=bu3[:, 0:1])
            nc.sync.dma_start(out=outT[...], in_=osb[...])

    nc.compile()
    return nc


def kernel(x, rnn_input, deltas, lengths, x_dnn_params, u_dnn_params, W_ih, W_hh,
           b_rnn):
    in_maps, meta = prep(x, rnn_input, deltas, lengths, x_dnn_params, u_dnn_params,
                         W_ih, W_hh, b_rnn)
    nc = build(meta["S"], meta["T"], meta["Ns"])
    res = run_bass_kernel_spmd(nc, in_maps, core_ids=list(range(N_CORES)))
    B = rnn_input.shape[0]
    out = np.zeros((B, STATE), dtype=np.float32)
    for j in range(N_CORES):
        out[meta["idx"][:, j]] = res.results[j]["outT"].T
    coefficients = np.zeros((2, 2), dtype=np.float32)
    return out, coefficients


# revision 2
# speedup vs baseline: 1.5316x; 1.5316x over previous
"""CausalFlowModel TRN2 kernel v2: bf16 datapath for the scan, fp32 encoder/decoder/d-path."""
import sys

sys.path.insert(0, "/opt/trn_rl_repo")
import numpy as np
import ml_dtypes
import concourse.bass as bass
import concourse.bacc as bacc
import concourse.mybir as mybir
from concourse.tile import TileContext
from concourse.bass_utils import run_bass_kernel_spmd

F32 = mybir.dt.float32
BF16 = mybir.dt.bfloat16
I32 = mybir.dt.int32
AF = mybir.ActivationFunctionType
ALU = mybir.AluOpType
NPBF = ml_dtypes.bfloat16

N_CORES = 8
STATE, CTRL, RNN, OUT = 16, 8, 64, 16
Z = RNN + STATE  # 80
KU = CTRL + 1  # 9
KAUG = Z + KU + 1  # 90
X_DNN_H = 2 * RNN  # 128
U_DNN_H = 2 * Z  # 160

ZPERM = np.concatenate([np.arange(STATE, Z), np.arange(0, STATE)])


def _np(a):
    return np.asarray(a, dtype=np.float32)


def prep(x, rnn_input, deltas, lengths, x_dnn_params, u_dnn_params, W_ih, W_hh, b_rnn):
    # (doc) Ns = padded active widths; Cs = min-core guaranteed-active widths
    B, T = rnn_input.shape[0], rnn_input.shape[1]
    assert B % N_CORES == 0
    S = B // N_CORES
    x = _np(x); rnn_input = _np(rnn_input); deltas = _np(deltas)
    lengths = np.asarray(lengths, dtype=np.int64)

    order = np.argsort(-lengths, kind="stable")
    idx = order.reshape(S, N_CORES)

    lens_sorted = lengths[order]
    percore = lens_sorted.reshape(S, N_CORES)
    Ns, Cs = [], []
    for t in range(T):
        cnt = (percore > t).sum(axis=0)
        Ns.append(min(S, max(4, -(-int(cnt.max()) // 4) * 4)))
        Cs.append(int(cnt.min()))

    (W1, b1), (W2, b2), (W3, b3) = [(_np(w), _np(b)) for (w, b) in x_dnn_params]
    (W4, b4), (W5, b5), (W6, b6) = [(_np(w), _np(b)) for (w, b) in u_dnn_params]
    W_ih = _np(W_ih); W_hh = _np(W_hh); b_rnn = _np(b_rnn)

    colperm = np.concatenate([g * Z + ZPERM for g in range(4)])
    Whh_p = W_hh[np.ix_(ZPERM, colperm)]
    Wih_p = W_ih[:, colperm]
    b_p = b_rnn[colperm]
    Waug = np.concatenate([2.0 * Whh_p, Wih_p, b_p[None, :]], axis=0)
    Waug[:, 2 * Z:3 * Z] *= 2.0
    Waug = np.ascontiguousarray(Waug.astype(NPBF))

    W3h = np.ascontiguousarray(W3 * 0.5)
    b3h = np.ascontiguousarray((b3 * 0.5)[:, None])
    W4e = np.ascontiguousarray(2.0 * W4[ZPERM])

    shared = {
        "Wx1": np.ascontiguousarray(2.0 * W1), "bx1": np.ascontiguousarray(b1[:, None]),
        "Wx2": W2, "bx2": np.ascontiguousarray(b2[:, None]),
        "Wx3": W3h, "bx3": b3h,
        "Waug": Waug,
        "Wu1": W4e,
        "bu1a": np.ascontiguousarray(b4[:128, None]),
        "bu1b": np.ascontiguousarray(b4[128:, None]),
        "Wu2a": np.ascontiguousarray(W5[:128]), "Wu2b": np.ascontiguousarray(W5[128:]),
        "bu2a": np.ascontiguousarray(b5[:128, None]),
        "bu2b": np.ascontiguousarray(b5[128:, None]),
        "Wu3a": np.ascontiguousarray(W6[:128]), "Wu3b": np.ascontiguousarray(W6[128:]),
        "bu3": np.ascontiguousarray(b6[:, None]),
    }

    tgrid = np.arange(T)
    in_maps = []
    for j in range(N_CORES):
        sel = idx[:, j]
        lj = lengths[sel]
        xT_half = np.ascontiguousarray(0.5 * x[sel].T)  # [16,S] f32 (encoder)
        uT = np.ascontiguousarray(rnn_input[sel].transpose(1, 2, 0).astype(NPBF))
        dT = deltas[sel, :, 0].T.copy()
        dT = np.maximum(dT, 1e-30)
        dT[tgrid[:, None] >= lj[None, :]] = 0.0
        dT = np.ascontiguousarray(dT)
        dlast = np.ascontiguousarray(deltas[sel, lj - 1, 0][None, :])
        maskA = np.ascontiguousarray(
            np.broadcast_to(((lj % 2) == 1).astype(np.int32)[None, :], (Z, S)))
        m = dict(shared)
        m.update({"xT": xT_half, "uT": uT, "dT": dT, "dlast": dlast, "maskA": maskA})
        in_maps.append(m)

    meta = {"S": S, "T": T, "Ns": Ns, "Cs": Cs, "idx": idx}
    return in_maps, meta


def build(S, T, Ns, Cs=None):
    if Cs is None:
        Cs = [0] * T
    nc = bacc.Bacc("TRN2", target_bir_lowering=False, debug=False,
                   num_devices=N_CORES)
    d = {}
    def din(name, shape, dt=F32):
        d[name] = nc.dram_tensor(name, list(shape), dt, kind="ExternalInput")
    din("xT", (STATE, S)); din("uT", (T, KU, S), BF16); din("dT", (T, S))
    din("dlast", (1, S)); din("maskA", (Z, S), I32)
    din("Wx1", (STATE, X_DNN_H)); din("bx1", (X_DNN_H, 1))
    din("Wx2", (X_DNN_H, X_DNN_H)); din("bx2", (X_DNN_H, 1))
    din("Wx3", (X_DNN_H, RNN)); din("bx3", (RNN, 1))
    din("Waug", (KAUG, 4 * Z), BF16)
    din("Wu1", (Z, U_DNN_H)); din("bu1a", (128, 1)); din("bu1b", (U_DNN_H - 128, 1))
    din("Wu2a", (128, U_DNN_H)); din("Wu2b", (U_DNN_H - 128, U_DNN_H))
    din("bu2a", (128, 1)); din("bu2b", (U_DNN_H - 128, 1))
    din("Wu3a", (128, OUT)); din("Wu3b", (U_DNN_H - 128, OUT))
    din("bu3", (OUT, 1))
    outT = nc.dram_tensor("outT", [OUT, S], F32, kind="ExternalOutput")

    NUB = 14
    n_ugrp = -(-T // NUB)

    with TileContext(nc) as tc:
        with (
            tc.tile_pool(name="const", bufs=1) as cp,
            tc.tile_pool(name="state", bufs=1) as sp,
            tc.tile_pool(name="stage", bufs=4) as stp,
            tc.tile_pool(name="psum", bufs=1, space="PSUM") as pp,
            tc.tile_pool(name="psumd", bufs=2, space="PSUM") as ppd,
        ):
            def load(name, shape, dt=F32):
                t = cp.tile(list(shape), dt, tag=name)
                nc.sync.dma_start(out=t[...], in_=d[name][...])
                return t
            Wx1 = load("Wx1", (STATE, X_DNN_H)); bx1 = load("bx1", (X_DNN_H, 1))
            Wx2 = load("Wx2", (X_DNN_H, X_DNN_H)); bx2 = load("bx2", (X_DNN_H, 1))
            Wx3 = load("Wx3", (X_DNN_H, RNN)); bx3 = load("bx3", (RNN, 1))
            Waug = load("Waug", (KAUG, 4 * Z), BF16)
            Wu1 = load("Wu1", (Z, U_DNN_H))
            bu1a = load("bu1a", (128, 1)); bu1b = load("bu1b", (U_DNN_H - 128, 1))
            Wu2a = load("Wu2a", (128, U_DNN_H)); Wu2b = load("Wu2b", (U_DNN_H - 128, U_DNN_H))
            bu2a = load("bu2a", (128, 1)); bu2b = load("bu2b", (U_DNN_H - 128, 1))
            Wu3a = load("Wu3a", (128, OUT)); Wu3b = load("Wu3b", (U_DNN_H - 128, OUT))
            bu3 = load("bu3", (OUT, 1))
            maskA = load("maskA", (Z, S), I32)
            xsb = load("xT", (STATE, S))

            ustore = cp.tile([NUB * KU, n_ugrp, S], BF16, tag="ustore")
            for k in range(n_ugrp):
                t0, t1 = k * NUB, min((k + 1) * NUB, T)
                nc.sync.dma_start(
                    out=ustore[: (t1 - t0) * KU, k, :],
                    in_=d["uT"][t0:t1, :, :].rearrange("a b c -> (a b) c"),
                )

            ones = cp.tile([1, Z], F32, tag="ones")
            nc.vector.memset(ones[...], 1.0)

            rhsA = sp.tile([KAUG, S], BF16, tag="rhsA")
            rhsB = sp.tile([KAUG, S], BF16, tag="rhsB")
            rhs = [rhsA, rhsB]
            c = sp.tile([Z, S], BF16, tag="c")
            sg = sp.tile([Z, 4, S], BF16, tag="sg")
            sc = sp.tile([Z, S], BF16, tag="sc")
            a2t = sp.tile([Z, S], BF16, tag="a2t")
            t2t = sp.tile([Z, S], BF16, tag="t2t")
            ht2t = sp.tile([Z, S], BF16, tag="ht2t")
            dltt = sp.tile([Z, S], BF16, tag="dltt")
            mt = sp.tile([Z, S], BF16, tag="mt")
            htmp = sp.tile([Z, S], BF16, tag="htmp")

            nc.vector.memset(c[...], 0.0)
            onesrow = cp.tile([1, S], BF16, tag="onesrow")
            nc.vector.memset(onesrow[...], 1.0)
            nc.sync.dma_start(out=rhsA[Z + KU:, :], in_=onesrow[...])
            nc.sync.dma_start(out=rhsB[Z + KU:, :], in_=onesrow[...])

            # ---- encoder (fp32) -> z/2 in bf16 rhsB ----
            ep = pp.tile([X_DNN_H, S], F32, tag="mm128")
            nc.tensor.matmul(ep[...], Wx1[...], xsb[...], start=True, stop=True)
            e1 = sp.tile([X_DNN_H, S], F32, tag="enc1")
            nc.scalar.activation(e1[...], ep[...], AF.Tanh, bias=bx1[:, 0:1])
            ep2 = pp.tile([X_DNN_H, S], F32, tag="mm128")
            nc.tensor.matmul(ep2[...], Wx2[...], e1[...], start=True, stop=True)
            e2 = sp.tile([X_DNN_H, S], F32, tag="enc2")
            nc.scalar.activation(e2[...], ep2[...], AF.Tanh, bias=bx2[:, 0:1])
            ep3 = pp.tile([RNN, S], F32, tag="small")
            nc.tensor.matmul(ep3[...], Wx3[...], e2[...], start=True, stop=True)
            nc.scalar.activation(rhsB[:RNN, :], ep3[...], AF.Identity, bias=bx3[:, 0:1])
            xbf = sp.tile([STATE, S], BF16, tag="xbf")
            nc.vector.tensor_copy(xbf[...], xsb[...])
            nc.sync.dma_start(out=rhsB[RNN:Z, :], in_=xbf[...])

            SH = S // 2
            gatesA = pp.tile([Z, 4, SH], F32, tag="gatesA")
            gatesB = pp.tile([Z, 4, SH], F32, tag="gatesB")

            for t in range(T):
                N = Ns[t]
                NA = min(N, max(4, (-(-N // 8)) * 4))
                cur = rhs[(t + 1) % 2]
                nxt = rhs[t % 2]
                nc.sync.dma_start(
                    out=cur[Z:Z + KU, :N],
                    in_=ustore[(t % NUB) * KU:(t % NUB + 1) * KU, t // NUB, :N],
                )
                stg = stp.tile([1, S], F32, tag="dstage")
                nc.sync.dma_start(out=stg[:, :N], in_=d["dT"][t:t + 1, :N])
                dbc = ppd.tile([Z, S], F32, tag="dbc")
                nc.tensor.matmul(dbc[:, :N], ones[...], stg[:, :N], start=True, stop=True)
                halves = [(0, NA, gatesA), (NA, N, gatesB)] if NA < N else [(0, N, gatesA)]
                for (lo, hi, gt) in halves:
                    W = hi - lo
                    for g in range(4):
                        nc.tensor.matmul(gt[:, g, :W], Waug[:, g * Z:(g + 1) * Z],
                                         cur[:, lo:hi], start=True, stop=True)
                    nc.scalar.activation(sg[:, 0:3, lo:hi], gt[:, 0:3, :W], AF.Sigmoid)
                    nc.vector.scalar_tensor_tensor(
                        a2t[:, lo:hi], sg[:, 2, lo:hi], 0.5, sg[:, 0, lo:hi],
                        op0=ALU.subtract, op1=ALU.mult)
                    nc.vector.tensor_mul(t2t[:, lo:hi], sg[:, 1, lo:hi], c[:, lo:hi])
                    nc.vector.tensor_add(c[:, lo:hi], a2t[:, lo:hi], t2t[:, lo:hi])
                    nc.scalar.activation(sg[:, 3, lo:hi], gt[:, 3, :W], AF.Sigmoid)
                    nc.scalar.activation(sc[:, lo:hi], c[:, lo:hi], AF.Sigmoid, scale=4.0)
                    nc.vector.scalar_tensor_tensor(
                        ht2t[:, lo:hi], sc[:, lo:hi], 0.5, sg[:, 3, lo:hi],
                        op0=ALU.subtract, op1=ALU.mult)
                    nc.vector.tensor_sub(dltt[:, lo:hi], ht2t[:, lo:hi], cur[:Z, lo:hi])
                    nc.vector.tensor_mul(mt[:, lo:hi], dbc[:, lo:hi], dltt[:, lo:hi])
                    # columns < C are active on every core: write h directly.
                    # columns [C, hi) may be past end-of-sequence on this core:
                    # write via mask (d==0 freezes them).
                    Cc = max(lo, min(hi, Cs[t]))
                    if Cc > lo:
                        nc.vector.tensor_add(nxt[:Z, lo:Cc], cur[:Z, lo:Cc], mt[:, lo:Cc])
                    if hi > Cc:
                        nc.vector.tensor_add(htmp[:, Cc:hi], cur[:Z, Cc:hi], mt[:, Cc:hi])
                        nc.vector.copy_predicated(
                            nxt[:Z, Cc:hi], dbc[:, Cc:hi].bitcast(I32), htmp[:, Cc:hi])

            # ---- post-loop ----
            hlast = sp.tile([Z, S], BF16, tag="hlast")
            hprev = sp.tile([Z, S], BF16, tag="hprev")
            nc.vector.tensor_copy(hlast[...], rhsB[:Z, :])
            nc.vector.copy_predicated(hlast[...], maskA[...], rhsA[:Z, :])
            nc.vector.tensor_copy(hprev[...], rhsA[:Z, :])
            nc.vector.copy_predicated(hprev[...], maskA[...], rhsB[:Z, :])
            stg2 = stp.tile([1, S], F32, tag="dstage")
            nc.sync.dma_start(out=stg2[...], in_=d["dlast"][...])
            dbc2 = ppd.tile([Z, S], F32, tag="dbc")
            nc.tensor.matmul(dbc2[...], ones[...], stg2[...], start=True, stop=True)
            df1 = sp.tile([Z, S], F32, tag="df1")
            df2 = sp.tile([Z, S], F32, tag="df2")
            et = sp.tile([Z, S], F32, tag="et")
            nc.vector.tensor_sub(df1[...], hlast[...], hprev[...])
            nc.vector.tensor_mul(df2[...], dbc2[...], df1[...])
            nc.vector.tensor_add(et[...], hprev[...], df2[...])

            # ---- decoder (fp32) ----
            p1a = pp.tile([128, S], F32, tag="mm128")
            nc.tensor.matmul(p1a[...], Wu1[:, :128], et[...], start=True, stop=True)
            d1a = sp.tile([128, S], F32, tag="d1a")
            nc.scalar.activation(d1a[...], p1a[...], AF.Tanh, bias=bu1a[:, 0:1])
            p1b = pp.tile([U_DNN_H - 128, S], F32, tag="small")
            nc.tensor.matmul(p1b[...], Wu1[:, 128:], et[...], start=True, stop=True)
            d1b = sp.tile([U_DNN_H - 128, S], F32, tag="d1b")
            nc.scalar.activation(d1b[...], p1b[...], AF.Tanh, bias=bu1b[:, 0:1])

            p2a = pp.tile([128, S], F32, tag="mm128")
            nc.tensor.matmul(p2a[...], Wu2a[:, :128], d1a[...], start=True, stop=False)
            nc.tensor.matmul(p2a[...], Wu2b[:, :128], d1b[...], start=False, stop=True)
            d2a = sp.tile([128, S], F32, tag="d2a")
            nc.scalar.activation(d2a[...], p2a[...], AF.Tanh, bias=bu2a[:, 0:1])
            p2b = pp.tile([U_DNN_H - 128, S], F32, tag="small")
            nc.tensor.matmul(p2b[...], Wu2a[:, 128:], d1a[...], start=True, stop=False)
            nc.tensor.matmul(p2b[...], Wu2b[:, 128:], d1b[...], start=False, stop=True)
            d2b = sp.tile([U_DNN_H - 128, S], F32, tag="d2b")
            nc.scalar.activation(d2b[...], p2b[...], AF.Tanh, bias=bu2b[:, 0:1])

            p3 = pp.tile([OUT, S], F32, tag="small")
            nc.tensor.matmul(p3[...], Wu3a[...], d2a[...], start=True, stop=False)
            nc.tensor.matmul(p3[...], Wu3b[...], d2b[...], start=False, stop=True)
            osb = sp.tile([OUT, S], F32, tag="osb")
            nc.scalar.activation(osb[...], p3[...], AF.Identity, bias=bu3[:, 0:1])
            nc.sync.dma_start(out=outT[...], in_=osb[...])

    nc.compile()
    return nc


def kernel(x, rnn_input, deltas, lengths, x_dnn_params, u_dnn_params, W_ih, W_hh,
           b_rnn):
    in_maps, meta = prep(x, rnn_input, deltas, lengths, x_dnn_params, u_dnn_params,
                         W_ih, W_hh, b_rnn)
    nc = build(meta["S"], meta["T"], meta["Ns"], meta["Cs"])
    res = run_bass_kernel_spmd(nc, in_maps, core_ids=list(range(N_CORES)))
    B = rnn_input.shape[0]
    out = np.zeros((B, STATE), dtype=np.float32)
    for j in range(N_CORES):
        out[meta["idx"][:, j]] = res.results[j]["outT"].T
    coefficients = np.zeros((2, 2), dtype=np.float32)
    return out, coefficients
